# revision 2
# baseline (speedup 1.0000x reference)
"""Causal self-attention with sink logit on 8 Trainium2 NeuronCores.

nn_CausalSelfAttention: B=2, T=2048, C=1024, H=16, D=64.
    qkv = x @ w_qkv; per-head causal attention with a per-head sink logit in
    the softmax denominator; out = y @ w_proj.

Sharding: 8 cores = 2 batches x 4 head-groups (data-parallel over B,
tensor-parallel over heads). Each core computes its batch's qkv projection
restricted to its 4 heads, flash-style causal attention (S^T layout,
denominator via an appended ones-block in the V matmul, sink seeded into the
accumulator with a K=1 matmul), and the partial output projection against its
w_proj row-slice. The host transposes x per batch, pre-rounds all matmul
inputs to TF32 (the kernel runs the tensor engine in fp32r), and sums the 4
per-head-group partials per batch (the "all-reduce after c_proj", done on
host since the full output is assembled host-side anyway).

kernel(**inputs) takes the FULL unsharded inputs and returns the FULL output.
"""
from contextlib import ExitStack

import numpy as np

F32 = None
F32R = None

P_ = 128          # partitions
QB = 512          # psum bank width (fp32)
D = 64            # head dim
HPC = 4           # heads per core
NPAIR = 2
B, T, C, H = 2, 2048, 1024, 16
N_CORES = 8


def round_tf32(x):
    i = np.ascontiguousarray(x, dtype=np.float32).view(np.uint32).astype(np.uint64)
    lsb = (i >> 13) & 1
    i = i + 0x0FFF + lsb
    return (i & 0xFFFFE000).astype(np.uint32).view(np.float32)


def _build_bass():
    import concourse.mybir as mybir
    import concourse.tile as tile
    from concourse import bacc

    global F32, F32R
    F32 = mybir.dt.float32
    F32R = mybir.dt.float32r

    CCH = C // P_             # C chunks
    GW = min(QB, T // 2)      # q/t group width
    NG = T // GW              # groups
    NTCG = GW // P_           # t-chunks per group
    scale = 1.0 / np.sqrt(D)

    nc = bacc.Bacc("TRN2", target_bir_lowering=False, debug=False,
                   num_devices=N_CORES)

    xt_d = nc.dram_tensor("xt", [C, T], F32R, kind="ExternalInput")
    wqk_d = nc.dram_tensor("wqk", [C, 2 * HPC * D], F32R, kind="ExternalInput")
    wv_d = nc.dram_tensor("wv", [C, HPC * D], F32R, kind="ExternalInput")
    wproj_d = nc.dram_tensor("wproj", [HPC * D, C], F32R, kind="ExternalInput")
    es_d = nc.dram_tensor("esrows", [1, HPC * P_], F32R, kind="ExternalInput")
    ones_d = nc.dram_tensor("ones512", [1, QB], F32R, kind="ExternalInput")
    onesc_d = nc.dram_tensor("onesc", [P_, D], F32R, kind="ExternalInput")
    masks_d = nc.dram_tensor("masks", [4, P_, QB], F32R, kind="ExternalInput")
    out_d = nc.dram_tensor("out", [T, C], F32, kind="ExternalOutput")

    with tile.TileContext(nc) as tc, ExitStack() as ctx:
        pool = ctx.enter_context(tc.tile_pool(name="pool", bufs=1))
        xt_pool = ctx.enter_context(tc.tile_pool(name="xt", bufs=2 * CCH + 2))
        work = ctx.enter_context(tc.tile_pool(name="work", bufs=2))
        psum = ctx.enter_context(tc.tile_pool(name="ps", bufs=1, space="PSUM"))

        es = pool.tile([1, HPC * P_], F32R, tag="es")
        ones = pool.tile([1, QB], F32R, tag="ones")
        onesc = pool.tile([P_, D], F32R, tag="onesc")
        maskv = pool.tile([P_, 4, QB], F32R, tag="maskv")
        nc.sync.dma_start(es[:], es_d.ap())
        nc.sync.dma_start(ones[:], ones_d.ap())
        nc.sync.dma_start(onesc[:], onesc_d.ap())

        wqk = pool.tile([P_, CCH, 2 * HPC * D], F32R, tag="wqk")
        wv = pool.tile([P_, CCH, HPC * D], F32R, tag="wv")
        wproj = pool.tile([P_, 2, C], F32R, tag="wproj")
        for c in range(CCH):
            nc.sync.dma_start(wqk[:, c, :], wqk_d.ap()[c * P_:(c + 1) * P_, :])
            nc.sync.dma_start(wv[:, c, :], wv_d.ap()[c * P_:(c + 1) * P_, :])
        nc.sync.dma_start(wproj[:], wproj_d.ap().rearrange("(co ci) m -> ci co m", ci=P_))
        nc.sync.dma_start(maskv[:], masks_d.ap().rearrange("v p q -> p v q"))

        QKT = pool.tile([P_, 2 * NPAIR, T], F32R, tag="qkt")
        VO = pool.tile([P_, T // P_, HPC, P_], F32R, tag="vo")
        YT = pool.tile([P_, NPAIR, T], F32R, tag="yt")

        nc.vector.tensor_copy(
            VO[:, :, :, D:P_],
            onesc[:, None, None, :].to_broadcast([P_, T // P_, HPC, D]))
        for g in range(NG):
            tg0 = g * GW
            xg = [xt_pool.tile([P_, GW], F32R, tag="xt", name=f"x{g}_{c}")
                  for c in range(CCH)]
            for c in range(CCH):
                nc.scalar.dma_start(xg[c][:], xt_d.ap()[c * P_:(c + 1) * P_,
                                                        tg0:tg0 + GW])
            for m in range(2 * NPAIR):
                ps = psum.tile([P_, GW], F32, tag="qk", bufs=2, name=f"qk{g}_{m}")
                for c in range(CCH):
                    nc.tensor.matmul(
                        ps[:], wqk[:, c, m * P_:(m + 1) * P_], xg[c][:],
                        start=(c == 0), stop=(c == CCH - 1))
                nc.vector.tensor_copy(QKT[:, m, tg0:tg0 + GW], ps[:])
            for tcl in range(NTCG):
                tc_g = g * NTCG + tcl
                ps = psum.tile([P_, HPC * D], F32, tag="qk", bufs=2,
                               name=f"vps{g}_{tcl}")
                for c in range(CCH):
                    nc.tensor.matmul(
                        ps[:], xg[c][:, tcl * P_:(tcl + 1) * P_], wv[:, c, :],
                        start=(c == 0), stop=(c == CCH - 1))
                nc.vector.tensor_copy(
                    VO[:, tc_g, :, 0:D],
                    ps[:].rearrange("p (h d) -> p h d", h=HPC))

            kmax = (g + 1) * NTCG
            kdiag = g * NTCG
            for p in range(NPAIR):
                Y = [psum.tile([P_, QB], F32, tag=f"Y{e}",
                               name=f"Y{g}_{p}_{e}")[:, :GW]
                     for e in range(2)]
                for e in range(2):
                    h = 2 * p + e
                    nc.tensor.matmul(
                        Y[e][:], es[0:1, h * P_:(h + 1) * P_], ones[0:1, :GW],
                        start=True, stop=False)
                for kc in range(kmax):
                    S = psum.tile([P_, 2 * GW], F32, tag="S", bufs=2,
                                  name=f"S{g}_{p}_{kc}")
                    Pt = work.tile([P_, 2 * GW], F32R, tag="P", bufs=3,
                                   name=f"Pt{g}_{p}_{kc}")
                    for e in range(2):
                        rows = slice(D * e, D * e + D)
                        nc.tensor.matmul(
                            S[:, e * GW:(e + 1) * GW],
                            QKT[rows, 2 + p, kc * P_:(kc + 1) * P_],
                            QKT[rows, p, tg0:tg0 + GW],
                            start=True, stop=True)
                    nc.scalar.activation(
                        Pt[:], S[:], mybir.ActivationFunctionType.Exp,
                        scale=float(scale))
                    if kc >= kdiag:
                        v = kc - kdiag
                        w = P_ * (v + 1)
                        for e in range(2):
                            nc.vector.tensor_tensor(
                                Pt[:, e * GW:e * GW + w],
                                Pt[:, e * GW:e * GW + w],
                                maskv[:, v, :w], mybir.AluOpType.mult)
                    for e in range(2):
                        h = 2 * p + e
                        nc.tensor.matmul(
                            Y[e][:], VO[:, kc, h, :],
                            Pt[:, e * GW:(e + 1) * GW],
                            start=False, stop=(kc == kmax - 1))
                for e in range(2):
                    # cross-base mult needs one PSUM input (walrus rejects
                    # SBUF x SBUF with differing base partitions)
                    scr = work.tile([P_, GW], F32, tag="scr",
                                    name=f"scr{g}_{p}_{e}")
                    nc.vector.tensor_copy(scr[D:P_, :], Y[e][D:P_, :])
                    nc.vector.reciprocal(scr[D:P_, :], scr[D:P_, :])
                    nc.vector.tensor_tensor(
                        YT[D * e:D * e + D, p, tg0:tg0 + GW], Y[e][0:D, :],
                        scr[D:P_, :], mybir.AluOpType.mult)

            for tcl in range(g * NTCG, (g + 1) * NTCG):
                ob = work.tile([P_, C], F32, tag="ob", name=f"ob{tcl}")
                for nh in range(C // QB):
                    po = psum.tile([P_, QB], F32, tag="qk", bufs=2,
                                   name=f"po{tcl}_{nh}")
                    for cch in range(2):
                        nc.tensor.matmul(
                            po[:],
                            YT[:, cch, tcl * P_:(tcl + 1) * P_],
                            wproj[:, cch, nh * QB:(nh + 1) * QB],
                            start=(cch == 0), stop=(cch == 1))
                    if nh % 2 == 0:
                        nc.scalar.copy(ob[:, nh * QB:(nh + 1) * QB], po[:])
                    else:
                        nc.vector.tensor_copy(ob[:, nh * QB:(nh + 1) * QB], po[:])
                nc.sync.dma_start(out_d.ap()[tcl * P_:(tcl + 1) * P_, :], ob[:])

    nc.compile()
    return nc


def _make_core_inputs(x, w_qkv, w_proj, sink_logit, core):
    b, g = core // 4, core % 4
    h0 = g * HPC
    HD = H * D

    xt = round_tf32(np.ascontiguousarray(np.asarray(x[b], dtype=np.float32).T))
    wq = w_qkv[:, h0 * D:(h0 + HPC) * D]
    wk = w_qkv[:, HD + h0 * D: HD + (h0 + HPC) * D]
    wvv = w_qkv[:, 2 * HD + h0 * D: 2 * HD + (h0 + HPC) * D]
    wqk = round_tf32(np.ascontiguousarray(np.concatenate([wq, wk], axis=1)))
    wv = round_tf32(np.ascontiguousarray(wvv))
    wproj = round_tf32(np.ascontiguousarray(w_proj[h0 * D:(h0 + HPC) * D, :]))

    es = np.zeros((1, HPC * P_), np.float32)
    for hh in range(HPC):
        es[0, hh * P_ + D:(hh + 1) * P_] = np.exp(
            np.asarray(sink_logit[h0 + hh], dtype=np.float64)).astype(np.float32)
    es = round_tf32(es)

    masks = np.zeros((4, P_, QB), np.float32)
    for v in range(4):
        for k in range(P_):
            masks[v, k, 128 * v + k:] = 1.0

    return {
        "xt": xt, "wqk": wqk, "wv": wv, "wproj": wproj, "esrows": es,
        "ones512": np.ones((1, QB), np.float32),
        "onesc": np.ones((P_, D), np.float32),
        "masks": masks,
    }


_CACHE = {}


def _get_runner():
    """Build (once) the bass program and the jitted SPMD callable."""
    if "fn" in _CACHE:
        return _CACHE["fn"], _CACHE["meta"]

    import jax
    from jax.experimental.shard_map import shard_map
    from jax.sharding import Mesh, NamedSharding, PartitionSpec

    import concourse.mybir as mybir
    from concourse.bass2jax import (_bass_exec_p, install_neuronx_cc_hook,
                                    partition_id_tensor)

    nc = _build_bass()
    _CACHE["nc"] = nc
    install_neuronx_cc_hook()
    pid_name = nc.partition_id_tensor.name if nc.partition_id_tensor else None

    in_names, out_names, out_avals, zero_outs = [], [], [], []
    for alloc in nc.m.functions[0].allocations:
        if not isinstance(alloc, mybir.MemoryLocationSet):
            continue
        name = alloc.memorylocations[0].name
        if alloc.kind == "ExternalInput":
            if name != pid_name:
                in_names.append(name)
        elif alloc.kind == "ExternalOutput":
            out_names.append(name)
            shape = tuple(alloc.tensor_shape)
            dtype = mybir.dt.np(alloc.dtype)
            out_avals.append(jax.core.ShapedArray(shape, dtype))
            zero_outs.append(np.zeros(shape, dtype))
    n_params, n_outs = len(in_names), len(out_avals)
    all_names = in_names + out_names
    if pid_name is not None:
        all_names = all_names + [pid_name]

    def _body(*args):
        operands = list(args)
        if pid_name is not None:
            operands.append(partition_id_tensor())
        outs = _bass_exec_p.bind(
            *operands,
            out_avals=tuple(out_avals),
            in_names=tuple(all_names),
            out_names=tuple(out_names),
            lowering_input_output_aliases=(),
            sim_require_finite=True,
            sim_require_nnan=True,
            nc=nc,
        )
        return tuple(outs)

    devices = jax.devices()[:N_CORES]
    mesh = Mesh(np.asarray(devices), ("core",))
    spec = PartitionSpec("core")
    sharding = NamedSharding(mesh, spec)
    fn = jax.jit(
        shard_map(_body, mesh=mesh, in_specs=(spec,) * (n_params + n_outs),
                  out_specs=(spec,) * n_outs, check_rep=False),
        keep_unused=True)

    zeros_dev = [jax.device_put(
        np.zeros((N_CORES * z.shape[0], *z.shape[1:]), z.dtype), sharding)
        for z in zero_outs]

    meta = dict(in_names=in_names, out_names=out_names, out_avals=out_avals,
                sharding=sharding, zeros_dev=zeros_dev, jax=jax)
    _CACHE["fn"] = fn
    _CACHE["meta"] = meta
    return fn, meta


def kernel(x, w_qkv, w_proj, sink_logit):
    x = np.asarray(x, dtype=np.float32)
    w_qkv = np.asarray(w_qkv, dtype=np.float32)
    w_proj = np.asarray(w_proj, dtype=np.float32)
    sink_logit = np.asarray(sink_logit, dtype=np.float32)

    fn, meta = _get_runner()
    jax = meta["jax"]

    in_maps = [_make_core_inputs(x, w_qkv, w_proj, sink_logit, core)
               for core in range(N_CORES)]
    concat_in = [
        jax.device_put(
            np.concatenate([in_maps[c][nm] for c in range(N_CORES)], axis=0),
            meta["sharding"])
        for nm in meta["in_names"]]

    out_arrs = fn(*concat_in, *meta["zeros_dev"])
    jax.block_until_ready(out_arrs)

    i_out = meta["out_names"].index("out")
    per_core = np.asarray(out_arrs[i_out]).reshape(N_CORES, T, C)

    out = np.zeros((B, T, C), np.float64)
    for core in range(N_CORES):
        out[core // 4] += per_core[core].astype(np.float64)
    return out.astype(np.float32)



# revision 4
# speedup vs baseline: 1.3227x; 1.3227x over previous
"""Causal self-attention with sink logit on 8 Trainium2 NeuronCores.

nn_CausalSelfAttention: B=2, T=2048, C=1024, H=16, D=64.
    qkv = x @ w_qkv; per-head causal attention with a per-head sink logit in
    the softmax denominator; out = y @ w_proj.

Sharding: 8 cores = 2 batches x 4 head-groups (data-parallel over B,
tensor-parallel over heads). Each core computes its batch's qkv projection
restricted to its 4 heads, flash-style causal attention (S^T layout,
denominator via an appended ones-block in the V matmul, sink seeded into the
accumulator with a K=1 matmul), and the partial output projection against its
w_proj row-slice. Host converts inputs to bf16, transposes x per batch, and
sums the 4 per-head-group partials per batch.

Schedule: the whole per-core program is emitted as one software-pipelined
instruction stream. The attention inner loop (score matmul -> exp on the
activation engine -> PV matmul) is interleaved with "filler" matmuls (the
next group's qkv projection and completed groups' output projection) so the
tensor engine never stalls waiting for the exp, keeping it at the high
p-state clock. Output tiles are copied PSUM->SBUF on GpSimd and DMA'd out
from the GpSimd queue; QKT/VO copies and softmax normalize run on DVE.

kernel(**inputs) takes the FULL unsharded inputs and returns the FULL output.
"""
from collections import deque
from contextlib import ExitStack

import numpy as np

BF16 = None
F32 = None

P_ = 128          # partitions
GW = 512          # q/t group width
D = 64            # head dim
HPC = 4           # heads per core
NPAIR = 2
B, T, C, H = 2, 2048, 1024, 16
N_CORES = 8
CCH = C // P_     # 8 contraction chunks
NG = T // GW      # 4 groups
NTCG = GW // P_   # 4 t-chunks per group

PE_NS = 1.0 / 2.4  # ns per matmul row at full clock


def _build_bass():
    import concourse.mybir as mybir
    import concourse.tile as tile
    from concourse import bacc

    global BF16, F32
    F32 = mybir.dt.float32
    BF16 = mybir.dt.bfloat16

    scale = 1.0 / np.sqrt(D)

    nc = bacc.Bacc("TRN2", target_bir_lowering=False, debug=False,
                   num_devices=N_CORES)

    xt_d = nc.dram_tensor("xt", [C, T], BF16, kind="ExternalInput")
    wqk_d = nc.dram_tensor("wqk", [C, 2 * HPC * D], BF16, kind="ExternalInput")
    wv_d = nc.dram_tensor("wv", [C, HPC * D], BF16, kind="ExternalInput")
    wproj_d = nc.dram_tensor("wproj", [HPC * D, C], BF16, kind="ExternalInput")
    es_d = nc.dram_tensor("esrows", [1, HPC * P_], BF16, kind="ExternalInput")
    ones_d = nc.dram_tensor("ones512", [1, GW], BF16, kind="ExternalInput")
    onesc_d = nc.dram_tensor("onesc", [P_, D], BF16, kind="ExternalInput")
    masks_d = nc.dram_tensor("masks", [4, P_, GW], BF16, kind="ExternalInput")
    out_d = nc.dram_tensor("out", [T, C], BF16, kind="ExternalOutput")

    with tile.TileContext(nc) as tc, ExitStack() as ctx:
        pool = ctx.enter_context(tc.tile_pool(name="pool", bufs=1))
        xg_pool = ctx.enter_context(tc.tile_pool(name="xg", bufs=3))
        pt_pool = ctx.enter_context(tc.tile_pool(name="pt", bufs=3))
        ob_pool = ctx.enter_context(tc.tile_pool(name="ob", bufs=4))
        wk_pool = ctx.enter_context(tc.tile_pool(name="wk", bufs=2))
        psum = ctx.enter_context(tc.tile_pool(name="ps", bufs=1, space="PSUM"))

        es = pool.tile([1, HPC * P_], BF16, tag="es")
        ones1 = pool.tile([1, GW], BF16, tag="ones")
        onesc = pool.tile([P_, D], BF16, tag="onesc")
        maskv = pool.tile([P_, 4, GW], BF16, tag="maskv")
        wqk = pool.tile([P_, CCH, 2 * HPC * D], BF16, tag="wqk")
        wv = pool.tile([P_, CCH, HPC * D], BF16, tag="wv")
        wproj = pool.tile([P_, 2, C], BF16, tag="wproj")
        QKT = pool.tile([P_, 2 * NPAIR, T], BF16, tag="qkt")
        VO = pool.tile([P_, T // P_, HPC, P_], BF16, tag="vo")
        YT = pool.tile([P_, NPAIR, T], BF16, tag="yt")

        nc.sync.dma_start(es[:], es_d.ap())
        nc.sync.dma_start(ones1[:], ones_d.ap())
        nc.sync.dma_start(onesc[:], onesc_d.ap())
        nc.sync.dma_start(wqk[:], wqk_d.ap().rearrange("(co ci) m -> ci co m",
                                                       ci=P_))
        nc.sync.dma_start(wv[:], wv_d.ap().rearrange("(co ci) m -> ci co m",
                                                     ci=P_))
        nc.sync.dma_start(wproj[:], wproj_d.ap().rearrange(
            "(co ci) m -> ci co m", ci=P_))
        nc.sync.dma_start(maskv[:], masks_d.ap().rearrange("v p q -> p v q"))

        nc.vector.tensor_copy(
            VO[:, :, :, D:P_],
            onesc[:, None, None, :].to_broadcast([P_, T // P_, HPC, D]))

        xg = {}

        def dma_x(g):
            if g >= NG:
                return
            t = xg_pool.tile([P_, CCH, GW], BF16, tag="xt", name=f"x{g}")
            xg[g] = t
            nc.sync.dma_start(
                t[:],
                xt_d.ap()[:, g * GW:(g + 1) * GW].rearrange(
                    "(co ci) t -> ci co t", ci=P_))

        fill_q = deque()

        def emit_qk_unit(g, m):
            ps = psum.tile([P_, GW], F32, tag="G", bufs=2, name=f"qk{g}_{m}")
            for c in range(CCH):
                nc.tensor.matmul(ps[:], wqk[:, c, m * P_:(m + 1) * P_],
                                 xg[g][:, c, :],
                                 start=(c == 0), stop=(c == CCH - 1))
            nc.vector.tensor_copy(QKT[:, m, g * GW:(g + 1) * GW], ps[:])

        def emit_v_unit(g, tcl):
            ps = psum.tile([P_, GW], F32, tag="G", bufs=2, name=f"v{g}_{tcl}")
            pv = ps[:, :HPC * D]
            for c in range(CCH):
                nc.tensor.matmul(pv, xg[g][:, c, tcl * P_:(tcl + 1) * P_],
                                 wv[:, c, :],
                                 start=(c == 0), stop=(c == CCH - 1))
            nc.vector.tensor_copy(
                VO[:, g * NTCG + tcl, :, 0:D],
                pv.rearrange("p (h d) -> p h d", h=HPC))

        def emit_proj_unit(tcl, nh):
            po = psum.tile([P_, GW], F32, tag="G", bufs=2,
                           name=f"po{tcl}_{nh}")
            for cch in range(2):
                nc.tensor.matmul(po[:], YT[:, cch, tcl * P_:(tcl + 1) * P_],
                                 wproj[:, cch, nh * GW:(nh + 1) * GW],
                                 start=(cch == 0), stop=(cch == 1))
            ob = ob_pool.tile([P_, GW], BF16, tag="ob", name=f"ob{tcl}_{nh}")
            if nh % 2 == 0:
                nc.scalar.copy(ob[:], po[:])
            else:
                nc.vector.tensor_copy(ob[:], po[:])
            nc.gpsimd.dma_start(
                out_d.ap()[tcl * P_:(tcl + 1) * P_, nh * GW:(nh + 1) * GW],
                ob[:])

        def push_qkv(g):
            if g >= NG:
                return
            for m in range(2 * NPAIR):
                fill_q.append((lambda g=g, m=m: emit_qk_unit(g, m),
                               CCH * GW * PE_NS))
            for tcl in range(NTCG):
                fill_q.append((lambda g=g, tcl=tcl: emit_v_unit(g, tcl),
                               CCH * HPC * D * PE_NS))

        def push_proj(g):
            for tcl in range(g * NTCG, (g + 1) * NTCG):
                for nh in range(C // GW):
                    fill_q.append(
                        (lambda tcl=tcl, nh=nh: emit_proj_unit(tcl, nh),
                         2 * GW * PE_NS))

        def drain(ns):
            done = 0.0
            while fill_q and done < ns:
                fn, cost = fill_q.popleft()
                fn()
                done += cost

        def drain_all():
            while fill_q:
                fn, _ = fill_q.popleft()
                fn()

        # prologue: x for first two groups; first group's qkv inline
        dma_x(0)
        dma_x(1)
        for m in range(2 * NPAIR):
            emit_qk_unit(0, m)
        for tcl in range(NTCG):
            emit_v_unit(0, tcl)

        for g in range(NG):
            tg0 = g * GW
            dma_x(g + 2)
            push_qkv(g + 1)
            kmax = (g + 1) * NTCG
            kdiag = g * NTCG
            for p in range(NPAIR):
                Y = [psum.tile([P_, GW], F32, tag=f"Y{e}", name=f"Y{g}_{p}_{e}")
                     for e in range(2)]
                for e in range(2):
                    h = 2 * p + e
                    nc.tensor.matmul(Y[e][:], es[0:1, h * P_:(h + 1) * P_],
                                     ones1[0:1, :], start=True, stop=False)

                def emit_pv(item, p=p, Y=Y, kmax=kmax):
                    kc, Pt = item
                    for e in range(2):
                        h = 2 * p + e
                        nc.tensor.matmul(Y[e][:], VO[:, kc, h, :],
                                         Pt[:, e * GW:(e + 1) * GW],
                                         start=False, stop=(kc == kmax - 1))

                pv_pending = deque()
                for kc in range(kmax):
                    S = psum.tile([P_, 2 * GW], F32, tag="S", bufs=2,
                                  name=f"S{g}_{p}_{kc}")
                    Pt = pt_pool.tile([P_, 2 * GW], BF16, tag="P",
                                      name=f"Pt{g}_{p}_{kc}")
                    for e in range(2):
                        rows = slice(D * e, D * e + D)
                        nc.tensor.matmul(
                            S[:, e * GW:(e + 1) * GW],
                            QKT[rows, 2 + p, kc * P_:(kc + 1) * P_],
                            QKT[rows, p, tg0:tg0 + GW],
                            start=True, stop=True)
                    nc.scalar.activation(Pt[:], S[:],
                                         mybir.ActivationFunctionType.Exp,
                                         scale=float(scale))
                    if kc >= kdiag:
                        v = kc - kdiag
                        w = P_ * (v + 1)
                        for e in range(2):
                            nc.vector.tensor_tensor(
                                Pt[:, e * GW:e * GW + w],
                                Pt[:, e * GW:e * GW + w],
                                maskv[:, v, :w], mybir.AluOpType.mult)
                        drain(700)
                    else:
                        drain(450)
                    if pv_pending:
                        emit_pv(pv_pending.popleft())
                    pv_pending.append((kc, Pt))
                drain(600)
                while pv_pending:
                    emit_pv(pv_pending.popleft())
                    if pv_pending:
                        drain(600)
                for e in range(2):
                    scr = wk_pool.tile([P_, GW], F32, tag="scr",
                                       name=f"scr{g}_{p}_{e}")
                    nc.vector.reciprocal(scr[D:P_, :], Y[e][D:P_, :])
                    nc.vector.tensor_tensor(
                        YT[D * e:D * e + D, p, tg0:tg0 + GW], Y[e][0:D, :],
                        scr[D:P_, :], mybir.AluOpType.mult)
                drain(900)
            push_proj(g)
        drain_all()

    nc.compile()
    return nc


def _make_core_inputs(x, w_qkv, w_proj, sink_logit, core):
    import ml_dtypes
    BF = ml_dtypes.bfloat16

    b, g = core // 4, core % 4
    h0 = g * HPC
    HD = H * D

    xt = np.ascontiguousarray(
        np.asarray(x[b], dtype=np.float32).T).astype(BF)
    wq = w_qkv[:, h0 * D:(h0 + HPC) * D]
    wk = w_qkv[:, HD + h0 * D: HD + (h0 + HPC) * D]
    wvv = w_qkv[:, 2 * HD + h0 * D: 2 * HD + (h0 + HPC) * D]
    wqk = np.ascontiguousarray(
        np.concatenate([wq, wk], axis=1)).astype(BF)
    wv = np.ascontiguousarray(wvv).astype(BF)
    wproj = np.ascontiguousarray(w_proj[h0 * D:(h0 + HPC) * D, :]).astype(BF)

    es = np.zeros((1, HPC * P_), np.float32)
    for hh in range(HPC):
        es[0, hh * P_ + D:(hh + 1) * P_] = np.exp(
            np.asarray(sink_logit[h0 + hh], dtype=np.float64)).astype(
                np.float32)

    masks = np.zeros((4, P_, GW), np.float32)
    for v in range(4):
        for k in range(P_):
            masks[v, k, P_ * v + k:] = 1.0

    return {
        "xt": xt, "wqk": wqk, "wv": wv, "wproj": wproj,
        "esrows": es.astype(BF),
        "ones512": np.ones((1, GW), BF),
        "onesc": np.ones((P_, D), BF),
        "masks": masks.astype(BF),
    }


_CACHE = {}


def _get_runner():
    """Build (once) the bass program and the jitted SPMD callable."""
    if "fn" in _CACHE:
        return _CACHE["fn"], _CACHE["meta"]

    import jax
    from jax.experimental.shard_map import shard_map
    from jax.sharding import Mesh, NamedSharding, PartitionSpec

    import concourse.mybir as mybir
    from concourse.bass2jax import (_bass_exec_p, install_neuronx_cc_hook,
                                    partition_id_tensor)

    nc = _build_bass()
    _CACHE["nc"] = nc
    install_neuronx_cc_hook()
    pid_name = nc.partition_id_tensor.name if nc.partition_id_tensor else None

    in_names, out_names, out_avals, zero_outs = [], [], [], []
    for alloc in nc.m.functions[0].allocations:
        if not isinstance(alloc, mybir.MemoryLocationSet):
            continue
        name = alloc.memorylocations[0].name
        if alloc.kind == "ExternalInput":
            if name != pid_name:
                in_names.append(name)
        elif alloc.kind == "ExternalOutput":
            out_names.append(name)
            shape = tuple(alloc.tensor_shape)
            dtype = mybir.dt.np(alloc.dtype)
            out_avals.append(jax.core.ShapedArray(shape, dtype))
            zero_outs.append(np.zeros(shape, dtype))
    n_params, n_outs = len(in_names), len(out_avals)
    all_names = in_names + out_names
    if pid_name is not None:
        all_names = all_names + [pid_name]

    def _body(*args):
        operands = list(args)
        if pid_name is not None:
            operands.append(partition_id_tensor())
        outs = _bass_exec_p.bind(
            *operands,
            out_avals=tuple(out_avals),
            in_names=tuple(all_names),
            out_names=tuple(out_names),
            lowering_input_output_aliases=(),
            sim_require_finite=True,
            sim_require_nnan=True,
            nc=nc,
        )
        return tuple(outs)

    devices = jax.devices()[:N_CORES]
    mesh = Mesh(np.asarray(devices), ("core",))
    spec = PartitionSpec("core")
    sharding = NamedSharding(mesh, spec)
    fn = jax.jit(
        shard_map(_body, mesh=mesh, in_specs=(spec,) * (n_params + n_outs),
                  out_specs=(spec,) * n_outs, check_rep=False),
        keep_unused=True)

    zeros_dev = [jax.device_put(
        np.zeros((N_CORES * z.shape[0], *z.shape[1:]), z.dtype), sharding)
        for z in zero_outs]

    meta = dict(in_names=in_names, out_names=out_names, out_avals=out_avals,
                sharding=sharding, zeros_dev=zeros_dev, jax=jax)
    _CACHE["fn"] = fn
    _CACHE["meta"] = meta
    return fn, meta


def kernel(x, w_qkv, w_proj, sink_logit):
    x = np.asarray(x, dtype=np.float32)
    w_qkv = np.asarray(w_qkv, dtype=np.float32)
    w_proj = np.asarray(w_proj, dtype=np.float32)
    sink_logit = np.asarray(sink_logit, dtype=np.float32)

    fn, meta = _get_runner()
    jax = meta["jax"]

    in_maps = [_make_core_inputs(x, w_qkv, w_proj, sink_logit, core)
               for core in range(N_CORES)]
    concat_in = [
        jax.device_put(
            np.concatenate([in_maps[c][nm] for c in range(N_CORES)], axis=0),
            meta["sharding"])
        for nm in meta["in_names"]]

    out_arrs = fn(*concat_in, *meta["zeros_dev"])
    jax.block_until_ready(out_arrs)

    i_out = meta["out_names"].index("out")
    per_core = np.asarray(out_arrs[i_out]).reshape(N_CORES, T, C)

    out = np.zeros((B, T, C), np.float64)
    for core in range(N_CORES):
        out[core // 4] += per_core[core].astype(np.float64)
    return out.astype(np.float32)


# revision 10
# speedup vs baseline: 1.3623x; 1.0300x over previous
"""Causal self-attention with sink logit on 8 Trainium2 NeuronCores.

nn_CausalSelfAttention: B=2, T=2048, C=1024, H=16, D=64.
    qkv = x @ w_qkv; per-head causal attention with a per-head sink logit in
    the softmax denominator; out = y @ w_proj.

Sharding: 8 cores = 2 batches x 4 head-groups (data-parallel over B,
tensor-parallel over heads). Each core computes its batch's qkv projection
restricted to its 4 heads, flash-style causal attention (S^T layout,
denominator via an appended ones-block in the V matmul, sink seeded into the
accumulator with a K=1 matmul), and the partial output projection against its
w_proj row-slice. Host converts inputs to bf16, transposes x per batch, and
sums the 4 per-head-group partials per batch.

Schedule: the whole per-core program is emitted as one software-pipelined
instruction stream. The attention inner loop (score matmul -> exp on the
activation engine -> PV matmul) is interleaved with "filler" matmuls (the
next group's qkv projection and completed groups' output projection) so the
tensor engine never stalls waiting for the exp, keeping it at the high
p-state clock. Output tiles are copied PSUM->SBUF on GpSimd and DMA'd out
from the GpSimd queue; QKT/VO copies and softmax normalize run on DVE.

kernel(**inputs) takes the FULL unsharded inputs and returns the FULL output.
"""
from collections import deque
from contextlib import ExitStack

import numpy as np

BF16 = None
F32 = None

P_ = 128          # partitions
GW = 512          # q/t group width
D = 64            # head dim
HPC = 4           # heads per core
NPAIR = 2
B, T, C, H = 2, 2048, 1024, 16
N_CORES = 8
CCH = C // P_     # 8 contraction chunks
NG = T // GW      # 4 groups
NTCG = GW // P_   # 4 t-chunks per group

PE_NS = 1.0 / 2.4  # ns per matmul row at full clock


def _build_bass():
    import concourse.mybir as mybir
    import concourse.tile as tile
    from concourse import bacc

    global BF16, F32
    F32 = mybir.dt.float32
    BF16 = mybir.dt.bfloat16

    scale = 1.0 / np.sqrt(D)

    nc = bacc.Bacc("TRN2", target_bir_lowering=False, debug=False,
                   num_devices=N_CORES)

    xt_d = nc.dram_tensor("xt", [C, T], BF16, kind="ExternalInput")
    wqk_d = nc.dram_tensor("wqk", [C, 2 * HPC * D], BF16, kind="ExternalInput")
    wv_d = nc.dram_tensor("wv", [C, HPC * D], BF16, kind="ExternalInput")
    wproj_d = nc.dram_tensor("wproj", [HPC * D, C], BF16, kind="ExternalInput")
    es_d = nc.dram_tensor("esrows", [1, HPC * P_], BF16, kind="ExternalInput")
    ones_d = nc.dram_tensor("ones512", [1, GW], BF16, kind="ExternalInput")
    onesc_d = nc.dram_tensor("onesc", [P_, D], BF16, kind="ExternalInput")
    masks_d = nc.dram_tensor("masks", [4, P_, GW], BF16, kind="ExternalInput")
    out_d = nc.dram_tensor("out", [T, C], BF16, kind="ExternalOutput")

    with tile.TileContext(nc) as tc, ExitStack() as ctx:
        pool = ctx.enter_context(tc.tile_pool(name="pool", bufs=1))
        xg_pool = ctx.enter_context(tc.tile_pool(name="xg", bufs=3))
        pt_pool = ctx.enter_context(tc.tile_pool(name="pt", bufs=3))
        ob_pool = ctx.enter_context(tc.tile_pool(name="ob", bufs=4))
        wk_pool = ctx.enter_context(tc.tile_pool(name="wk", bufs=2))
        psum = ctx.enter_context(tc.tile_pool(name="ps", bufs=1, space="PSUM"))

        es = pool.tile([1, HPC * P_], BF16, tag="es")
        ones1 = pool.tile([1, GW], BF16, tag="ones")
        onesc = pool.tile([P_, D], BF16, tag="onesc")
        maskv = pool.tile([P_, 4, GW], BF16, tag="maskv")
        wqk = pool.tile([P_, CCH, 2 * HPC * D], BF16, tag="wqk")
        wv = pool.tile([P_, CCH, HPC * D], BF16, tag="wv")
        wproj = pool.tile([P_, 2, C], BF16, tag="wproj")
        QKT = pool.tile([P_, 2 * NPAIR, T], BF16, tag="qkt")
        VO = pool.tile([P_, T // P_, HPC, P_], BF16, tag="vo")
        YT = pool.tile([P_, NPAIR, T], BF16, tag="yt")

        nc.sync.dma_start(es[:], es_d.ap())
        nc.sync.dma_start(ones1[:], ones_d.ap())
        nc.sync.dma_start(onesc[:], onesc_d.ap())
        nc.sync.dma_start(wqk[:], wqk_d.ap().rearrange("(co ci) m -> ci co m",
                                                       ci=P_))
        nc.sync.dma_start(wv[:], wv_d.ap().rearrange("(co ci) m -> ci co m",
                                                     ci=P_))
        nc.sync.dma_start(wproj[:], wproj_d.ap().rearrange(
            "(co ci) m -> ci co m", ci=P_))
        nc.sync.dma_start(maskv[:], masks_d.ap().rearrange("v p q -> p v q"))

        nc.vector.tensor_copy(
            VO[:, :, :, D:P_],
            onesc[:, None, None, :].to_broadcast([P_, T // P_, HPC, D]))

        xg = {}

        def dma_x(g):
            if g >= NG:
                return
            t = xg_pool.tile([P_, CCH, GW], BF16, tag="xt", name=f"x{g}")
            xg[g] = t
            nc.sync.dma_start(
                t[:],
                xt_d.ap()[:, g * GW:(g + 1) * GW].rearrange(
                    "(co ci) t -> ci co t", ci=P_))

        fill_q = deque()
        debt = [0.0]

        def emit_qk_unit(g, m):
            ps = psum.tile([P_, GW], F32, tag="G", bufs=2, name=f"qk{g}_{m}")
            for c in range(CCH):
                nc.tensor.matmul(ps[:], wqk[:, c, m * P_:(m + 1) * P_],
                                 xg[g][:, c, :],
                                 start=(c == 0), stop=(c == CCH - 1))
            nc.vector.tensor_copy(QKT[:, m, g * GW:(g + 1) * GW], ps[:])

        def emit_v_unit(g, tcl):
            ps = psum.tile([P_, GW], F32, tag="G", bufs=2, name=f"v{g}_{tcl}")
            pv = ps[:, :HPC * D]
            for c in range(CCH):
                nc.tensor.matmul(pv, xg[g][:, c, tcl * P_:(tcl + 1) * P_],
                                 wv[:, c, :],
                                 start=(c == 0), stop=(c == CCH - 1))
            nc.vector.tensor_copy(
                VO[:, g * NTCG + tcl, :, 0:D],
                pv.rearrange("p (h d) -> p h d", h=HPC))

        def emit_proj_unit(tcl, nh):
            po = psum.tile([P_, GW], F32, tag="G", bufs=2,
                           name=f"po{tcl}_{nh}")
            for cch in range(2):
                nc.tensor.matmul(po[:], YT[:, cch, tcl * P_:(tcl + 1) * P_],
                                 wproj[:, cch, nh * GW:(nh + 1) * GW],
                                 start=(cch == 0), stop=(cch == 1))
            ob = ob_pool.tile([P_, GW], BF16, tag="ob", name=f"ob{tcl}_{nh}")
            nc.vector.tensor_copy(ob[:], po[:])
            nc.gpsimd.dma_start(
                out_d.ap()[tcl * P_:(tcl + 1) * P_, nh * GW:(nh + 1) * GW],
                ob[:])

        def qkv_units(g):
            if g >= NG:
                return []
            u = [(lambda g=g, m=m: emit_qk_unit(g, m), CCH * GW * PE_NS)
                 for m in range(2 * NPAIR)]
            u += [(lambda g=g, tcl=tcl: emit_v_unit(g, tcl),
                   CCH * HPC * D * PE_NS) for tcl in range(NTCG)]
            return u

        def proj_units(g):
            if g < 0:
                return []
            return [(lambda tcl=tcl, nh=nh: emit_proj_unit(tcl, nh),
                     2 * GW * PE_NS)
                    for tcl in range(g * NTCG, (g + 1) * NTCG)
                    for nh in range(C // GW)]

        def push_interleaved(a, b):
            # alternate long qkv units with short proj units so PSUM bank
            # release never gates two short units back-to-back
            out = []
            while a or b:
                if a:
                    out.append(a.pop(0))
                if b:
                    out.append(b.pop(0))
            fill_q.extend(out)

        def drain(ns):
            debt[0] += ns
            while fill_q and debt[0] >= fill_q[0][1]:
                fn, cost = fill_q.popleft()
                fn()
                debt[0] -= cost

        def drain_all():
            while fill_q:
                fn, _ = fill_q.popleft()
                fn()
            debt[0] = 0.0

        # prologue: x for first two groups; first group's qkv inline
        dma_x(0)
        dma_x(1)
        for m in range(2 * NPAIR):
            emit_qk_unit(0, m)
        for tcl in range(NTCG):
            emit_v_unit(0, tcl)

        for g in range(NG):
            tg0 = g * GW
            dma_x(g + 2)
            push_interleaved(qkv_units(g + 1), proj_units(g - 1))
            kmax = (g + 1) * NTCG
            kdiag = g * NTCG
            for p in range(NPAIR):
                Y = [psum.tile([P_, GW], F32, tag=f"Y{e}", name=f"Y{g}_{p}_{e}")
                     for e in range(2)]
                for e in range(2):
                    h = 2 * p + e
                    nc.tensor.matmul(Y[e][:], es[0:1, h * P_:(h + 1) * P_],
                                     ones1[0:1, :], start=True, stop=False)

                def emit_pv(item, p=p, Y=Y):
                    # kc descends; kc == 0 closes the accumulation
                    kc, Pt, c0 = item
                    for e in range(2):
                        h = 2 * p + e
                        nc.tensor.matmul(Y[e][:, c0:GW], VO[:, kc, h, :],
                                         Pt[:, e * GW + c0:(e + 1) * GW],
                                         start=False, stop=(kc == 0))

                pv_pending = deque()
                prev_masked = False
                # descending kc: restricted diagonal blocks first, full-width
                # kc=0 last (carries the accumulation stop)
                for kc in range(kmax - 1, -1, -1):
                    v = kc - kdiag          # >= 0 on the diagonal
                    c0 = P_ * v if v > 0 else 0   # fully-masked cols skipped
                    S = psum.tile([P_, 2 * GW], F32, tag="S", bufs=2,
                                  name=f"S{g}_{p}_{kc}")
                    Pt = pt_pool.tile([P_, 2 * GW], BF16, tag="P",
                                      name=f"Pt{g}_{p}_{kc}")
                    for e in range(2):
                        rows = slice(D * e, D * e + D)
                        nc.tensor.matmul(
                            S[:, e * GW + c0:(e + 1) * GW],
                            QKT[rows, 2 + p, kc * P_:(kc + 1) * P_],
                            QKT[rows, p, tg0 + c0:tg0 + GW],
                            start=True, stop=True)
                    s_ns = 2 * (GW - c0) * PE_NS
                    sv = S[:].rearrange("pt (e q) -> pt e q", e=2)[:, :, c0:GW]
                    pv = Pt[:].rearrange("pt (e q) -> pt e q", e=2)[:, :, c0:GW]
                    nc.scalar.activation(pv, sv,
                                         mybir.ActivationFunctionType.Exp,
                                         scale=float(scale))
                    masked = v >= 0
                    if masked:
                        for e in range(2):
                            nc.vector.tensor_tensor(
                                Pt[:, e * GW + c0:e * GW + c0 + P_],
                                Pt[:, e * GW + c0:e * GW + c0 + P_],
                                maskv[:, v, c0:c0 + P_], mybir.AluOpType.mult)
                    drain(max(250.0,
                              (2000.0 if prev_masked else 1400.0) - s_ns))
                    prev_masked = masked
                    if pv_pending:
                        emit_pv(pv_pending.popleft())
                    pv_pending.append((kc, Pt, c0))
                drain(1500)
                while pv_pending:
                    emit_pv(pv_pending.popleft())
                    if pv_pending:
                        drain(800)
                for e in range(2):
                    scr = wk_pool.tile([P_, GW], F32, tag="scr",
                                       name=f"scr{g}_{p}_{e}")
                    nc.vector.reciprocal(scr[D:P_, :], Y[e][D:P_, :])
                    nc.vector.tensor_tensor(
                        YT[D * e:D * e + D, p, tg0:tg0 + GW], Y[e][0:D, :],
                        scr[D:P_, :], mybir.AluOpType.mult)
                drain(2200)
        drain_all()
        for fn, _ in proj_units(NG - 1):
            fn()

    nc.compile()
    return nc


def _make_core_inputs(x, w_qkv, w_proj, sink_logit, core):
    import ml_dtypes
    BF = ml_dtypes.bfloat16

    b, g = core // 4, core % 4
    h0 = g * HPC
    HD = H * D

    xt = np.ascontiguousarray(
        np.asarray(x[b], dtype=np.float32).T).astype(BF)
    wq = w_qkv[:, h0 * D:(h0 + HPC) * D]
    wk = w_qkv[:, HD + h0 * D: HD + (h0 + HPC) * D]
    wvv = w_qkv[:, 2 * HD + h0 * D: 2 * HD + (h0 + HPC) * D]
    wqk = np.ascontiguousarray(
        np.concatenate([wq, wk], axis=1)).astype(BF)
    wv = np.ascontiguousarray(wvv).astype(BF)
    wproj = np.ascontiguousarray(w_proj[h0 * D:(h0 + HPC) * D, :]).astype(BF)

    es = np.zeros((1, HPC * P_), np.float32)
    for hh in range(HPC):
        es[0, hh * P_ + D:(hh + 1) * P_] = np.exp(
            np.asarray(sink_logit[h0 + hh], dtype=np.float64)).astype(
                np.float32)

    masks = np.zeros((4, P_, GW), np.float32)
    for v in range(4):
        for k in range(P_):
            masks[v, k, P_ * v + k:] = 1.0

    return {
        "xt": xt, "wqk": wqk, "wv": wv, "wproj": wproj,
        "esrows": es.astype(BF),
        "ones512": np.ones((1, GW), BF),
        "onesc": np.ones((P_, D), BF),
        "masks": masks.astype(BF),
    }


_CACHE = {}


def _get_runner():
    """Build (once) the bass program and the jitted SPMD callable."""
    if "fn" in _CACHE:
        return _CACHE["fn"], _CACHE["meta"]

    import jax
    from jax.experimental.shard_map import shard_map
    from jax.sharding import Mesh, NamedSharding, PartitionSpec

    import concourse.mybir as mybir
    from concourse.bass2jax import (_bass_exec_p, install_neuronx_cc_hook,
                                    partition_id_tensor)

    nc = _build_bass()
    _CACHE["nc"] = nc
    install_neuronx_cc_hook()
    pid_name = nc.partition_id_tensor.name if nc.partition_id_tensor else None

    in_names, out_names, out_avals, zero_outs = [], [], [], []
    for alloc in nc.m.functions[0].allocations:
        if not isinstance(alloc, mybir.MemoryLocationSet):
            continue
        name = alloc.memorylocations[0].name
        if alloc.kind == "ExternalInput":
            if name != pid_name:
                in_names.append(name)
        elif alloc.kind == "ExternalOutput":
            out_names.append(name)
            shape = tuple(alloc.tensor_shape)
            dtype = mybir.dt.np(alloc.dtype)
            out_avals.append(jax.core.ShapedArray(shape, dtype))
            zero_outs.append(np.zeros(shape, dtype))
    n_params, n_outs = len(in_names), len(out_avals)
    all_names = in_names + out_names
    if pid_name is not None:
        all_names = all_names + [pid_name]

    def _body(*args):
        operands = list(args)
        if pid_name is not None:
            operands.append(partition_id_tensor())
        outs = _bass_exec_p.bind(
            *operands,
            out_avals=tuple(out_avals),
            in_names=tuple(all_names),
            out_names=tuple(out_names),
            lowering_input_output_aliases=(),
            sim_require_finite=True,
            sim_require_nnan=True,
            nc=nc,
        )
        return tuple(outs)

    devices = jax.devices()[:N_CORES]
    mesh = Mesh(np.asarray(devices), ("core",))
    spec = PartitionSpec("core")
    sharding = NamedSharding(mesh, spec)
    fn = jax.jit(
        shard_map(_body, mesh=mesh, in_specs=(spec,) * (n_params + n_outs),
                  out_specs=(spec,) * n_outs, check_rep=False),
        keep_unused=True)

    zeros_dev = [jax.device_put(
        np.zeros((N_CORES * z.shape[0], *z.shape[1:]), z.dtype), sharding)
        for z in zero_outs]

    meta = dict(in_names=in_names, out_names=out_names, out_avals=out_avals,
                sharding=sharding, zeros_dev=zeros_dev, jax=jax)
    _CACHE["fn"] = fn
    _CACHE["meta"] = meta
    return fn, meta


def kernel(x, w_qkv, w_proj, sink_logit):
    x = np.asarray(x, dtype=np.float32)
    w_qkv = np.asarray(w_qkv, dtype=np.float32)
    w_proj = np.asarray(w_proj, dtype=np.float32)
    sink_logit = np.asarray(sink_logit, dtype=np.float32)

    fn, meta = _get_runner()
    jax = meta["jax"]

    in_maps = [_make_core_inputs(x, w_qkv, w_proj, sink_logit, core)
               for core in range(N_CORES)]
    concat_in = [
        jax.device_put(
            np.concatenate([in_maps[c][nm] for c in range(N_CORES)], axis=0),
            meta["sharding"])
        for nm in meta["in_names"]]

    out_arrs = fn(*concat_in, *meta["zeros_dev"])
    jax.block_until_ready(out_arrs)

    i_out = meta["out_names"].index("out")
    per_core = np.asarray(out_arrs[i_out]).reshape(N_CORES, T, C)

    out = np.zeros((B, T, C), np.float64)
    for core in range(N_CORES):
        out[core // 4] += per_core[core].astype(np.float64)
    return out.astype(np.float32)


# revision 12
# speedup vs baseline: 1.4472x; 1.0623x over previous
"""Causal self-attention with sink logit on 8 Trainium2 NeuronCores.

nn_CausalSelfAttention: B=2, T=2048, C=1024, H=16, D=64.
    qkv = x @ w_qkv; per-head causal attention with a per-head sink logit in
    the softmax denominator; out = y @ w_proj.

Sharding: 8 cores = 2 batches x 4 head-groups (data-parallel over B,
tensor-parallel over heads). Each core computes its batch's qkv projection
restricted to its 4 heads, flash-style causal attention (S^T layout,
denominator via an appended ones-block in the V matmul, sink seeded into the
accumulator with a K=1 matmul), and the partial output projection against its
w_proj row-slice. Host converts inputs to bf16, transposes x per batch, and
sums the 4 per-head-group partials per batch.

Schedule: the whole per-core program is emitted as one software-pipelined
instruction stream. The attention inner loop (score matmul -> exp on the
activation engine -> PV matmul) is interleaved with "filler" matmuls (the
next group's qkv projection and completed groups' output projection) so the
tensor engine never stalls waiting for the exp, keeping it at the high
p-state clock. Output tiles are copied PSUM->SBUF on GpSimd and DMA'd out
from the GpSimd queue; QKT/VO copies and softmax normalize run on DVE.

kernel(**inputs) takes the FULL unsharded inputs and returns the FULL output.
"""
from collections import deque
from contextlib import ExitStack

import numpy as np

BF16 = None
F32 = None

P_ = 128          # partitions
GW = 512          # q/t group width
D = 64            # head dim
HPC = 4           # heads per core
NPAIR = 2
B, T, C, H = 2, 2048, 1024, 16
N_CORES = 8
CCH = C // P_     # 8 contraction chunks
NG = T // GW      # 4 groups
NTCG = GW // P_   # 4 t-chunks per group

PE_NS = 1.0 / 2.4  # ns per matmul row at full clock


def _build_bass():
    import concourse.mybir as mybir
    import concourse.tile as tile
    from concourse import bacc

    global BF16, F32
    F32 = mybir.dt.float32
    BF16 = mybir.dt.bfloat16

    scale = 1.0 / np.sqrt(D)

    nc = bacc.Bacc("TRN2", target_bir_lowering=False, debug=False,
                   num_devices=N_CORES)

    xt_d = nc.dram_tensor("xt", [C, T], BF16, kind="ExternalInput")
    wqk_d = nc.dram_tensor("wqk", [C, 2 * HPC * D], BF16, kind="ExternalInput")
    wv_d = nc.dram_tensor("wv", [C, HPC * D], BF16, kind="ExternalInput")
    wproj_d = nc.dram_tensor("wproj", [HPC * D, C], BF16, kind="ExternalInput")
    es_d = nc.dram_tensor("esrows", [1, HPC * P_], BF16, kind="ExternalInput")
    ones_d = nc.dram_tensor("ones512", [1, GW], BF16, kind="ExternalInput")
    onesc_d = nc.dram_tensor("onesc", [P_, D], BF16, kind="ExternalInput")
    masks_d = nc.dram_tensor("masks", [4, P_, GW], BF16, kind="ExternalInput")
    out_d = nc.dram_tensor("out", [T, C], BF16, kind="ExternalOutput")

    with tile.TileContext(nc) as tc, ExitStack() as ctx:
        pool = ctx.enter_context(tc.tile_pool(name="pool", bufs=1))
        xg_pool = ctx.enter_context(tc.tile_pool(name="xg", bufs=3))
        pt_pool = ctx.enter_context(tc.tile_pool(name="pt", bufs=3))
        ob_pool = ctx.enter_context(tc.tile_pool(name="ob", bufs=4))
        wk_pool = ctx.enter_context(tc.tile_pool(name="wk", bufs=2))
        psum = ctx.enter_context(tc.tile_pool(name="ps", bufs=1, space="PSUM"))

        es = pool.tile([1, HPC * P_], BF16, tag="es")
        ones1 = pool.tile([1, GW], BF16, tag="ones")
        onesc = pool.tile([P_, D], BF16, tag="onesc")
        maskv = pool.tile([P_, 4, GW], BF16, tag="maskv")
        wqk = pool.tile([P_, CCH, 2 * HPC * D], BF16, tag="wqk")
        wv = pool.tile([P_, CCH, HPC * D], BF16, tag="wv")
        wproj = pool.tile([P_, 2, C], BF16, tag="wproj")
        QKT = pool.tile([P_, 2 * NPAIR, T], BF16, tag="qkt")
        VO = pool.tile([P_, T // P_, HPC, P_], BF16, tag="vo")
        YT = pool.tile([P_, NPAIR, T], BF16, tag="yt")

        nc.sync.dma_start(es[:], es_d.ap())
        nc.sync.dma_start(ones1[:], ones_d.ap())
        nc.sync.dma_start(onesc[:], onesc_d.ap())
        nc.sync.dma_start(wqk[:], wqk_d.ap().rearrange("(co ci) m -> ci co m",
                                                       ci=P_))
        nc.sync.dma_start(wv[:], wv_d.ap().rearrange("(co ci) m -> ci co m",
                                                     ci=P_))
        nc.sync.dma_start(wproj[:], wproj_d.ap().rearrange(
            "(co ci) m -> ci co m", ci=P_))
        nc.sync.dma_start(maskv[:], masks_d.ap().rearrange("v p q -> p v q"))

        nc.vector.tensor_copy(
            VO[:, :, :, D:P_],
            onesc[:, None, None, :].to_broadcast([P_, T // P_, HPC, D]))

        xg = {}

        def dma_x(g):
            if g >= NG:
                return
            t = xg_pool.tile([P_, CCH, GW], BF16, tag="xt", name=f"x{g}")
            xg[g] = t
            nc.sync.dma_start(
                t[:],
                xt_d.ap()[:, g * GW:(g + 1) * GW].rearrange(
                    "(co ci) t -> ci co t", ci=P_))

        fill_q = deque()
        debt = [0.0]

        def emit_qk_unit(g, m):
            ps = psum.tile([P_, GW], F32, tag="G", bufs=2, name=f"qk{g}_{m}")
            for c in range(CCH):
                nc.tensor.matmul(ps[:], wqk[:, c, m * P_:(m + 1) * P_],
                                 xg[g][:, c, :],
                                 start=(c == 0), stop=(c == CCH - 1))
            nc.vector.tensor_copy(QKT[:, m, g * GW:(g + 1) * GW], ps[:])

        def emit_v_unit(g, tcl):
            ps = psum.tile([P_, GW], F32, tag="G", bufs=2, name=f"v{g}_{tcl}")
            pv = ps[:, :HPC * D]
            for c in range(CCH):
                nc.tensor.matmul(pv, xg[g][:, c, tcl * P_:(tcl + 1) * P_],
                                 wv[:, c, :],
                                 start=(c == 0), stop=(c == CCH - 1))
            nc.vector.tensor_copy(
                VO[:, g * NTCG + tcl, :, 0:D],
                pv.rearrange("p (h d) -> p h d", h=HPC))

        def emit_proj_unit(tcl, nh):
            po = psum.tile([P_, GW], F32, tag="G", bufs=2,
                           name=f"po{tcl}_{nh}")
            for cch in range(2):
                nc.tensor.matmul(po[:], YT[:, cch, tcl * P_:(tcl + 1) * P_],
                                 wproj[:, cch, nh * GW:(nh + 1) * GW],
                                 start=(cch == 0), stop=(cch == 1))
            ob = ob_pool.tile([P_, GW], BF16, tag="ob", name=f"ob{tcl}_{nh}")
            nc.vector.tensor_copy(ob[:], po[:])
            nc.gpsimd.dma_start(
                out_d.ap()[tcl * P_:(tcl + 1) * P_, nh * GW:(nh + 1) * GW],
                ob[:])

        def qkv_units(g):
            if g >= NG:
                return []
            u = [(lambda g=g, m=m: emit_qk_unit(g, m), CCH * GW * PE_NS)
                 for m in range(2 * NPAIR)]
            u += [(lambda g=g, tcl=tcl: emit_v_unit(g, tcl),
                   CCH * HPC * D * PE_NS) for tcl in range(NTCG)]
            return u

        def proj_units(g):
            if g < 0:
                return []
            return [(lambda tcl=tcl, nh=nh: emit_proj_unit(tcl, nh),
                     2 * GW * PE_NS)
                    for tcl in range(g * NTCG, (g + 1) * NTCG)
                    for nh in range(C // GW)]

        def push_interleaved(a, b):
            # alternate long qkv units with short proj units so PSUM bank
            # release never gates two short units back-to-back
            out = []
            while a or b:
                if a:
                    out.append(a.pop(0))
                if b:
                    out.append(b.pop(0))
            fill_q.extend(out)

        def drain(ns):
            debt[0] = min(debt[0] + ns, 2500.0)
            while fill_q and debt[0] >= fill_q[0][1]:
                fn, cost = fill_q.popleft()
                fn()
                debt[0] -= cost

        def drain_all():
            while fill_q:
                fn, _ = fill_q.popleft()
                fn()
            debt[0] = 0.0

        # prologue: x for first two groups; first group's qkv inline
        dma_x(0)
        dma_x(1)
        for m in range(2 * NPAIR):
            emit_qk_unit(0, m)
        for tcl in range(NTCG):
            emit_v_unit(0, tcl)

        for g in range(NG):
            tg0 = g * GW
            dma_x(g + 2)
            push_interleaved(qkv_units(g + 1), proj_units(g - 1))
            kmax = (g + 1) * NTCG
            kdiag = g * NTCG
            for p in range(NPAIR):
                Y = [psum.tile([P_, GW], F32, tag=f"Y{e}", name=f"Y{g}_{p}_{e}")
                     for e in range(2)]
                for e in range(2):
                    h = 2 * p + e
                    nc.tensor.matmul(Y[e][:], es[0:1, h * P_:(h + 1) * P_],
                                     ones1[0:1, :], start=True, stop=False)

                def emit_pv(item, p=p, Y=Y):
                    # kc descends; kc == 0 closes the accumulation
                    kc, Pt, c0 = item
                    for e in range(2):
                        h = 2 * p + e
                        nc.tensor.matmul(Y[e][:, c0:GW], VO[:, kc, h, :],
                                         Pt[:, e * GW + c0:(e + 1) * GW],
                                         start=False, stop=(kc == 0))

                pv_pending = deque()
                prev_masked = False
                # descending kc: restricted diagonal blocks first, full-width
                # kc=0 last (carries the accumulation stop)
                for kc in range(kmax - 1, -1, -1):
                    v = kc - kdiag          # >= 0 on the diagonal
                    c0 = P_ * v if v > 0 else 0   # fully-masked cols skipped
                    S = psum.tile([P_, 2 * GW], F32, tag="S", bufs=2,
                                  name=f"S{g}_{p}_{kc}")
                    Pt = pt_pool.tile([P_, 2 * GW], BF16, tag="P",
                                      name=f"Pt{g}_{p}_{kc}")
                    for e in range(2):
                        rows = slice(D * e, D * e + D)
                        nc.tensor.matmul(
                            S[:, e * GW + c0:(e + 1) * GW],
                            QKT[rows, 2 + p, kc * P_:(kc + 1) * P_],
                            QKT[rows, p, tg0 + c0:tg0 + GW],
                            start=True, stop=True)
                    s_ns = 2 * (GW - c0) * PE_NS
                    sv = S[:].rearrange("pt (e q) -> pt e q", e=2)[:, :, c0:GW]
                    pv = Pt[:].rearrange("pt (e q) -> pt e q", e=2)[:, :, c0:GW]
                    nc.scalar.activation(pv, sv,
                                         mybir.ActivationFunctionType.Exp,
                                         scale=float(scale))
                    masked = v >= 0
                    if masked:
                        for e in range(2):
                            nc.vector.tensor_tensor(
                                Pt[:, e * GW + c0:e * GW + c0 + P_],
                                Pt[:, e * GW + c0:e * GW + c0 + P_],
                                maskv[:, v, c0:c0 + P_], mybir.AluOpType.mult)
                    drain(max(250.0,
                              (2000.0 if prev_masked else 1400.0) - s_ns))
                    prev_masked = masked
                    if pv_pending:
                        emit_pv(pv_pending.popleft())
                    pv_pending.append((kc, Pt, c0))
                drain(1500)
                while pv_pending:
                    emit_pv(pv_pending.popleft())
                    if pv_pending:
                        drain(800)
                # normalize: 1/denom = exp(-ln(denom)) on ACT (no table
                # reload: ln+exp share one act table). Copy the numerator
                # half out on DVE first so the Y banks release ~700ns after
                # the last PV instead of after the whole recip chain.
                for e in range(2):
                    yc = wk_pool.tile([D, GW], F32, tag="yc",
                                      name=f"yc{g}_{p}_{e}")
                    lnt = wk_pool.tile([D, GW], F32, tag="ln",
                                       name=f"ln{g}_{p}_{e}")
                    rnt = wk_pool.tile([D, GW], F32, tag="rn",
                                       name=f"rn{g}_{p}_{e}")
                    nc.vector.tensor_copy(yc[:], Y[e][0:D, :])
                    nc.scalar.activation(lnt[:], Y[e][D:P_, :],
                                         mybir.ActivationFunctionType.Ln)
                    nc.scalar.activation(rnt[:], lnt[:],
                                         mybir.ActivationFunctionType.Exp,
                                         scale=-1.0)
                    nc.vector.tensor_tensor(
                        YT[D * e:D * e + D, p, tg0:tg0 + GW], yc[:],
                        rnt[:], mybir.AluOpType.mult)
                drain(1400)
        drain_all()
        for fn, _ in proj_units(NG - 1):
            fn()

    nc.compile()
    return nc


def _make_core_inputs(x, w_qkv, w_proj, sink_logit, core):
    import ml_dtypes
    BF = ml_dtypes.bfloat16

    b, g = core // 4, core % 4
    h0 = g * HPC
    HD = H * D

    xt = np.ascontiguousarray(
        np.asarray(x[b], dtype=np.float32).T).astype(BF)
    wq = w_qkv[:, h0 * D:(h0 + HPC) * D]
    wk = w_qkv[:, HD + h0 * D: HD + (h0 + HPC) * D]
    wvv = w_qkv[:, 2 * HD + h0 * D: 2 * HD + (h0 + HPC) * D]
    wqk = np.ascontiguousarray(
        np.concatenate([wq, wk], axis=1)).astype(BF)
    wv = np.ascontiguousarray(wvv).astype(BF)
    wproj = np.ascontiguousarray(w_proj[h0 * D:(h0 + HPC) * D, :]).astype(BF)

    es = np.zeros((1, HPC * P_), np.float32)
    for hh in range(HPC):
        es[0, hh * P_ + D:(hh + 1) * P_] = np.exp(
            np.asarray(sink_logit[h0 + hh], dtype=np.float64)).astype(
                np.float32)

    masks = np.zeros((4, P_, GW), np.float32)
    for v in range(4):
        for k in range(P_):
            masks[v, k, P_ * v + k:] = 1.0

    return {
        "xt": xt, "wqk": wqk, "wv": wv, "wproj": wproj,
        "esrows": es.astype(BF),
        "ones512": np.ones((1, GW), BF),
        "onesc": np.ones((P_, D), BF),
        "masks": masks.astype(BF),
    }


_CACHE = {}


def _get_runner():
    """Build (once) the bass program and the jitted SPMD callable."""
    if "fn" in _CACHE:
        return _CACHE["fn"], _CACHE["meta"]

    import jax
    from jax.experimental.shard_map import shard_map
    from jax.sharding import Mesh, NamedSharding, PartitionSpec

    import concourse.mybir as mybir
    from concourse.bass2jax import (_bass_exec_p, install_neuronx_cc_hook,
                                    partition_id_tensor)

    nc = _build_bass()
    _CACHE["nc"] = nc
    install_neuronx_cc_hook()
    pid_name = nc.partition_id_tensor.name if nc.partition_id_tensor else None

    in_names, out_names, out_avals, zero_outs = [], [], [], []
    for alloc in nc.m.functions[0].allocations:
        if not isinstance(alloc, mybir.MemoryLocationSet):
            continue
        name = alloc.memorylocations[0].name
        if alloc.kind == "ExternalInput":
            if name != pid_name:
                in_names.append(name)
        elif alloc.kind == "ExternalOutput":
            out_names.append(name)
            shape = tuple(alloc.tensor_shape)
            dtype = mybir.dt.np(alloc.dtype)
            out_avals.append(jax.core.ShapedArray(shape, dtype))
            zero_outs.append(np.zeros(shape, dtype))
    n_params, n_outs = len(in_names), len(out_avals)
    all_names = in_names + out_names
    if pid_name is not None:
        all_names = all_names + [pid_name]

    def _body(*args):
        operands = list(args)
        if pid_name is not None:
            operands.append(partition_id_tensor())
        outs = _bass_exec_p.bind(
            *operands,
            out_avals=tuple(out_avals),
            in_names=tuple(all_names),
            out_names=tuple(out_names),
            lowering_input_output_aliases=(),
            sim_require_finite=True,
            sim_require_nnan=True,
            nc=nc,
        )
        return tuple(outs)

    devices = jax.devices()[:N_CORES]
    mesh = Mesh(np.asarray(devices), ("core",))
    spec = PartitionSpec("core")
    sharding = NamedSharding(mesh, spec)
    fn = jax.jit(
        shard_map(_body, mesh=mesh, in_specs=(spec,) * (n_params + n_outs),
                  out_specs=(spec,) * n_outs, check_rep=False),
        keep_unused=True)

    zeros_dev = [jax.device_put(
        np.zeros((N_CORES * z.shape[0], *z.shape[1:]), z.dtype), sharding)
        for z in zero_outs]

    meta = dict(in_names=in_names, out_names=out_names, out_avals=out_avals,
                sharding=sharding, zeros_dev=zeros_dev, jax=jax)
    _CACHE["fn"] = fn
    _CACHE["meta"] = meta
    return fn, meta


def kernel(x, w_qkv, w_proj, sink_logit):
    x = np.asarray(x, dtype=np.float32)
    w_qkv = np.asarray(w_qkv, dtype=np.float32)
    w_proj = np.asarray(w_proj, dtype=np.float32)
    sink_logit = np.asarray(sink_logit, dtype=np.float32)

    fn, meta = _get_runner()
    jax = meta["jax"]

    in_maps = [_make_core_inputs(x, w_qkv, w_proj, sink_logit, core)
               for core in range(N_CORES)]
    concat_in = [
        jax.device_put(
            np.concatenate([in_maps[c][nm] for c in range(N_CORES)], axis=0),
            meta["sharding"])
        for nm in meta["in_names"]]

    out_arrs = fn(*concat_in, *meta["zeros_dev"])
    jax.block_until_ready(out_arrs)

    i_out = meta["out_names"].index("out")
    per_core = np.asarray(out_arrs[i_out]).reshape(N_CORES, T, C)

    out = np.zeros((B, T, C), np.float64)
    for core in range(N_CORES):
        out[core // 4] += per_core[core].astype(np.float64)
    return out.astype(np.float32)


# revision 13
# speedup vs baseline: 1.6444x; 1.1362x over previous
"""Causal self-attention with sink logit on 8 Trainium2 NeuronCores.

nn_CausalSelfAttention: B=2, T=2048, C=1024, H=16, D=64.
    qkv = x @ w_qkv; per-head causal attention with a per-head sink logit in
    the softmax denominator; out = y @ w_proj.

Sharding: 8 cores = 2 batches x 4 head-groups (data-parallel over B,
tensor-parallel over heads). Each core computes its batch's qkv projection
restricted to its 4 heads, flash-style causal attention (S^T layout,
denominator via an appended ones-block in the V matmul, sink seeded into the
accumulator with a K=1 matmul), and the partial output projection against its
w_proj row-slice. Host converts inputs to bf16, transposes x per batch, and
sums the 4 per-head-group partials per batch.

Schedule: the whole per-core program is emitted as one software-pipelined
instruction stream. The attention inner loop (score matmul -> exp on the
activation engine -> PV matmul) is interleaved with "filler" matmuls (the
next group's qkv projection and completed groups' output projection) so the
tensor engine never stalls waiting for the exp, keeping it at the high
p-state clock. Output tiles are copied PSUM->SBUF on GpSimd and DMA'd out
from the GpSimd queue; QKT/VO copies and softmax normalize run on DVE.

kernel(**inputs) takes the FULL unsharded inputs and returns the FULL output.
"""
from collections import deque
from contextlib import ExitStack

import numpy as np

BF16 = None
F32 = None

P_ = 128          # partitions
GW = 512          # q/t group width
D = 64            # head dim
HPC = 4           # heads per core
NPAIR = 2
B, T, C, H = 2, 2048, 1024, 16
N_CORES = 8
CCH = C // P_     # 8 contraction chunks
NG = T // GW      # 4 groups
NTCG = GW // P_   # 4 t-chunks per group

PE_NS = 1.0 / 2.4  # ns per matmul row at full clock


def _patch_act_tables():
    """Steer every activation to the one table that holds exp+ln+copy.

    bacc's table-load pass assigns each activation the first table in
    act_info order that contains its function; exp alone matches a table
    without ln, so an exp/ln mix ping-pongs tables (1.3us reload each).
    Emptying the other exp-bearing sets (canonical indices preserved) makes
    exp, ln, and copy all resolve to natural_log_exp_and_others.
    """
    from concourse import bacc, hw_specs

    if getattr(bacc, "_act_tables_patched", False):
        return
    orig = hw_specs.get_activation_tables

    def patched(arch):
        tabs = dict(orig(arch))
        if "natural_log_exp_and_others" in tabs:
            keep = tabs["natural_log_exp_and_others"]
            for name in list(tabs):
                if name != "natural_log_exp_and_others" and (tabs[name] & keep):
                    tabs[name] = tabs[name] - keep
        return tabs

    bacc.get_activation_tables = patched
    bacc._act_tables_patched = True


def _build_bass():
    import concourse.mybir as mybir
    import concourse.tile as tile
    from concourse import bacc

    _patch_act_tables()

    global BF16, F32
    F32 = mybir.dt.float32
    BF16 = mybir.dt.bfloat16

    scale = 1.0 / np.sqrt(D)

    nc = bacc.Bacc("TRN2", target_bir_lowering=False, debug=False,
                   num_devices=N_CORES)

    xt_d = nc.dram_tensor("xt", [C, T], BF16, kind="ExternalInput")
    wqk_d = nc.dram_tensor("wqk", [C, 2 * HPC * D], BF16, kind="ExternalInput")
    wv_d = nc.dram_tensor("wv", [C, HPC * D], BF16, kind="ExternalInput")
    wproj_d = nc.dram_tensor("wproj", [HPC * D, C], BF16, kind="ExternalInput")
    es_d = nc.dram_tensor("esrows", [1, HPC * P_], BF16, kind="ExternalInput")
    ones_d = nc.dram_tensor("ones512", [1, GW], BF16, kind="ExternalInput")
    onesc_d = nc.dram_tensor("onesc", [P_, D], BF16, kind="ExternalInput")
    masks_d = nc.dram_tensor("masks", [4, P_, GW], BF16, kind="ExternalInput")
    out_d = nc.dram_tensor("out", [T, C], BF16, kind="ExternalOutput")

    with tile.TileContext(nc) as tc, ExitStack() as ctx:
        pool = ctx.enter_context(tc.tile_pool(name="pool", bufs=1))
        xg_pool = ctx.enter_context(tc.tile_pool(name="xg", bufs=3))
        pt_pool = ctx.enter_context(tc.tile_pool(name="pt", bufs=3))
        ob_pool = ctx.enter_context(tc.tile_pool(name="ob", bufs=4))
        wk_pool = ctx.enter_context(tc.tile_pool(name="wk", bufs=2))
        psum = ctx.enter_context(tc.tile_pool(name="ps", bufs=1, space="PSUM"))

        es = pool.tile([1, HPC * P_], BF16, tag="es")
        ones1 = pool.tile([1, GW], BF16, tag="ones")
        onesc = pool.tile([P_, D], BF16, tag="onesc")
        maskv = pool.tile([P_, 4, GW], BF16, tag="maskv")
        wqk = pool.tile([P_, CCH, 2 * HPC * D], BF16, tag="wqk")
        wv = pool.tile([P_, CCH, HPC * D], BF16, tag="wv")
        wproj = pool.tile([P_, 2, C], BF16, tag="wproj")
        QKT = pool.tile([P_, 2 * NPAIR, T], BF16, tag="qkt")
        VO = pool.tile([P_, T // P_, HPC, P_], BF16, tag="vo")
        YT = pool.tile([P_, NPAIR, T], BF16, tag="yt")

        nc.sync.dma_start(es[:], es_d.ap())
        nc.sync.dma_start(ones1[:], ones_d.ap())
        nc.sync.dma_start(onesc[:], onesc_d.ap())
        nc.sync.dma_start(wqk[:], wqk_d.ap().rearrange("(co ci) m -> ci co m",
                                                       ci=P_))
        nc.sync.dma_start(wv[:], wv_d.ap().rearrange("(co ci) m -> ci co m",
                                                     ci=P_))
        nc.sync.dma_start(wproj[:], wproj_d.ap().rearrange(
            "(co ci) m -> ci co m", ci=P_))
        nc.sync.dma_start(maskv[:], masks_d.ap().rearrange("v p q -> p v q"))

        nc.vector.tensor_copy(
            VO[:, :, :, D:P_],
            onesc[:, None, None, :].to_broadcast([P_, T // P_, HPC, D]))

        xg = {}

        def dma_x(g):
            if g >= NG:
                return
            t = xg_pool.tile([P_, CCH, GW], BF16, tag="xt", name=f"x{g}")
            xg[g] = t
            nc.sync.dma_start(
                t[:],
                xt_d.ap()[:, g * GW:(g + 1) * GW].rearrange(
                    "(co ci) t -> ci co t", ci=P_))

        fill_q = deque()
        debt = [0.0]

        def emit_qk_unit(g, m):
            ps = psum.tile([P_, GW], F32, tag="G", bufs=2, name=f"qk{g}_{m}")
            for c in range(CCH):
                nc.tensor.matmul(ps[:], wqk[:, c, m * P_:(m + 1) * P_],
                                 xg[g][:, c, :],
                                 start=(c == 0), stop=(c == CCH - 1))
            nc.vector.tensor_copy(QKT[:, m, g * GW:(g + 1) * GW], ps[:])

        def emit_v_unit(g, tcl):
            ps = psum.tile([P_, GW], F32, tag="G", bufs=2, name=f"v{g}_{tcl}")
            pv = ps[:, :HPC * D]
            for c in range(CCH):
                nc.tensor.matmul(pv, xg[g][:, c, tcl * P_:(tcl + 1) * P_],
                                 wv[:, c, :],
                                 start=(c == 0), stop=(c == CCH - 1))
            nc.vector.tensor_copy(
                VO[:, g * NTCG + tcl, :, 0:D],
                pv.rearrange("p (h d) -> p h d", h=HPC))

        def emit_proj_unit(tcl, nh):
            po = psum.tile([P_, GW], F32, tag="G", bufs=2,
                           name=f"po{tcl}_{nh}")
            for cch in range(2):
                nc.tensor.matmul(po[:], YT[:, cch, tcl * P_:(tcl + 1) * P_],
                                 wproj[:, cch, nh * GW:(nh + 1) * GW],
                                 start=(cch == 0), stop=(cch == 1))
            ob = ob_pool.tile([P_, GW], BF16, tag="ob", name=f"ob{tcl}_{nh}")
            nc.vector.tensor_copy(ob[:], po[:])
            nc.gpsimd.dma_start(
                out_d.ap()[tcl * P_:(tcl + 1) * P_, nh * GW:(nh + 1) * GW],
                ob[:])

        def qkv_units(g):
            if g >= NG:
                return []
            u = [(lambda g=g, m=m: emit_qk_unit(g, m), CCH * GW * PE_NS)
                 for m in range(2 * NPAIR)]
            u += [(lambda g=g, tcl=tcl: emit_v_unit(g, tcl),
                   CCH * HPC * D * PE_NS) for tcl in range(NTCG)]
            return u

        def proj_units(g):
            if g < 0:
                return []
            return [(lambda tcl=tcl, nh=nh: emit_proj_unit(tcl, nh),
                     2 * GW * PE_NS)
                    for tcl in range(g * NTCG, (g + 1) * NTCG)
                    for nh in range(C // GW)]

        def push_interleaved(a, b):
            # alternate long qkv units with short proj units so PSUM bank
            # release never gates two short units back-to-back
            out = []
            while a or b:
                if a:
                    out.append(a.pop(0))
                if b:
                    out.append(b.pop(0))
            fill_q.extend(out)

        def drain(ns):
            debt[0] = min(debt[0] + ns, 2500.0)
            while fill_q and debt[0] >= fill_q[0][1]:
                fn, cost = fill_q.popleft()
                fn()
                debt[0] -= cost

        def drain_all():
            while fill_q:
                fn, _ = fill_q.popleft()
                fn()
            debt[0] = 0.0

        # prologue: x for first two groups; first group's qkv inline
        dma_x(0)
        dma_x(1)
        for m in range(2 * NPAIR):
            emit_qk_unit(0, m)
        for tcl in range(NTCG):
            emit_v_unit(0, tcl)

        for g in range(NG):
            tg0 = g * GW
            dma_x(g + 2)
            push_interleaved(qkv_units(g + 1), proj_units(g - 1))
            kmax = (g + 1) * NTCG
            kdiag = g * NTCG
            for p in range(NPAIR):
                Y = [psum.tile([P_, GW], F32, tag=f"Y{e}", name=f"Y{g}_{p}_{e}")
                     for e in range(2)]
                for e in range(2):
                    h = 2 * p + e
                    nc.tensor.matmul(Y[e][:], es[0:1, h * P_:(h + 1) * P_],
                                     ones1[0:1, :], start=True, stop=False)

                def emit_pv(item, p=p, Y=Y):
                    # kc descends; kc == 0 closes the accumulation
                    kc, Pt, c0 = item
                    for e in range(2):
                        h = 2 * p + e
                        nc.tensor.matmul(Y[e][:, c0:GW], VO[:, kc, h, :],
                                         Pt[:, e * GW + c0:(e + 1) * GW],
                                         start=False, stop=(kc == 0))

                pv_pending = deque()
                prev_masked = False
                # descending kc: restricted diagonal blocks first, full-width
                # kc=0 last (carries the accumulation stop)
                for kc in range(kmax - 1, -1, -1):
                    v = kc - kdiag          # >= 0 on the diagonal
                    c0 = P_ * v if v > 0 else 0   # fully-masked cols skipped
                    S = psum.tile([P_, 2 * GW], F32, tag="S", bufs=2,
                                  name=f"S{g}_{p}_{kc}")
                    Pt = pt_pool.tile([P_, 2 * GW], BF16, tag="P",
                                      name=f"Pt{g}_{p}_{kc}")
                    for e in range(2):
                        rows = slice(D * e, D * e + D)
                        nc.tensor.matmul(
                            S[:, e * GW + c0:(e + 1) * GW],
                            QKT[rows, 2 + p, kc * P_:(kc + 1) * P_],
                            QKT[rows, p, tg0 + c0:tg0 + GW],
                            start=True, stop=True)
                    s_ns = 2 * (GW - c0) * PE_NS
                    sv = S[:].rearrange("pt (e q) -> pt e q", e=2)[:, :, c0:GW]
                    pv = Pt[:].rearrange("pt (e q) -> pt e q", e=2)[:, :, c0:GW]
                    nc.scalar.activation(pv, sv,
                                         mybir.ActivationFunctionType.Exp,
                                         scale=float(scale))
                    masked = v >= 0
                    if masked:
                        for e in range(2):
                            nc.vector.tensor_tensor(
                                Pt[:, e * GW + c0:e * GW + c0 + P_],
                                Pt[:, e * GW + c0:e * GW + c0 + P_],
                                maskv[:, v, c0:c0 + P_], mybir.AluOpType.mult)
                    drain(max(250.0,
                              (2000.0 if prev_masked else 1400.0) - s_ns))
                    prev_masked = masked
                    if pv_pending:
                        emit_pv(pv_pending.popleft())
                    pv_pending.append((kc, Pt, c0))
                drain(1500)
                while pv_pending:
                    emit_pv(pv_pending.popleft())
                    if pv_pending:
                        drain(800)
                # normalize: 1/denom = exp(-ln(denom)) on ACT (no table
                # reload: ln+exp share one act table). Copy the numerator
                # half out on DVE first so the Y banks release ~700ns after
                # the last PV instead of after the whole recip chain.
                for e in range(2):
                    yc = wk_pool.tile([D, GW], F32, tag="yc",
                                      name=f"yc{g}_{p}_{e}")
                    lnt = wk_pool.tile([D, GW], F32, tag="ln",
                                       name=f"ln{g}_{p}_{e}")
                    rnt = wk_pool.tile([D, GW], F32, tag="rn",
                                       name=f"rn{g}_{p}_{e}")
                    nc.vector.tensor_copy(yc[:], Y[e][0:D, :])
                    nc.scalar.activation(lnt[:], Y[e][D:P_, :],
                                         mybir.ActivationFunctionType.Ln)
                    nc.scalar.activation(rnt[:], lnt[:],
                                         mybir.ActivationFunctionType.Exp,
                                         scale=-1.0)
                    nc.vector.tensor_tensor(
                        YT[D * e:D * e + D, p, tg0:tg0 + GW], yc[:],
                        rnt[:], mybir.AluOpType.mult)
                drain(1400)
        drain_all()
        for fn, _ in proj_units(NG - 1):
            fn()

    nc.compile()
    return nc


def _make_core_inputs(x, w_qkv, w_proj, sink_logit, core):
    import ml_dtypes
    BF = ml_dtypes.bfloat16

    b, g = core // 4, core % 4
    h0 = g * HPC
    HD = H * D

    xt = np.ascontiguousarray(
        np.asarray(x[b], dtype=np.float32).T).astype(BF)
    wq = w_qkv[:, h0 * D:(h0 + HPC) * D]
    wk = w_qkv[:, HD + h0 * D: HD + (h0 + HPC) * D]
    wvv = w_qkv[:, 2 * HD + h0 * D: 2 * HD + (h0 + HPC) * D]
    wqk = np.ascontiguousarray(
        np.concatenate([wq, wk], axis=1)).astype(BF)
    wv = np.ascontiguousarray(wvv).astype(BF)
    wproj = np.ascontiguousarray(w_proj[h0 * D:(h0 + HPC) * D, :]).astype(BF)

    es = np.zeros((1, HPC * P_), np.float32)
    for hh in range(HPC):
        es[0, hh * P_ + D:(hh + 1) * P_] = np.exp(
            np.asarray(sink_logit[h0 + hh], dtype=np.float64)).astype(
                np.float32)

    masks = np.zeros((4, P_, GW), np.float32)
    for v in range(4):
        for k in range(P_):
            masks[v, k, P_ * v + k:] = 1.0

    return {
        "xt": xt, "wqk": wqk, "wv": wv, "wproj": wproj,
        "esrows": es.astype(BF),
        "ones512": np.ones((1, GW), BF),
        "onesc": np.ones((P_, D), BF),
        "masks": masks.astype(BF),
    }


_CACHE = {}


def _get_runner():
    """Build (once) the bass program and the jitted SPMD callable."""
    if "fn" in _CACHE:
        return _CACHE["fn"], _CACHE["meta"]

    import jax
    from jax.experimental.shard_map import shard_map
    from jax.sharding import Mesh, NamedSharding, PartitionSpec

    import concourse.mybir as mybir
    from concourse.bass2jax import (_bass_exec_p, install_neuronx_cc_hook,
                                    partition_id_tensor)

    nc = _build_bass()
    _CACHE["nc"] = nc
    install_neuronx_cc_hook()
    pid_name = nc.partition_id_tensor.name if nc.partition_id_tensor else None

    in_names, out_names, out_avals, zero_outs = [], [], [], []
    for alloc in nc.m.functions[0].allocations:
        if not isinstance(alloc, mybir.MemoryLocationSet):
            continue
        name = alloc.memorylocations[0].name
        if alloc.kind == "ExternalInput":
            if name != pid_name:
                in_names.append(name)
        elif alloc.kind == "ExternalOutput":
            out_names.append(name)
            shape = tuple(alloc.tensor_shape)
            dtype = mybir.dt.np(alloc.dtype)
            out_avals.append(jax.core.ShapedArray(shape, dtype))
            zero_outs.append(np.zeros(shape, dtype))
    n_params, n_outs = len(in_names), len(out_avals)
    all_names = in_names + out_names
    if pid_name is not None:
        all_names = all_names + [pid_name]

    def _body(*args):
        operands = list(args)
        if pid_name is not None:
            operands.append(partition_id_tensor())
        outs = _bass_exec_p.bind(
            *operands,
            out_avals=tuple(out_avals),
            in_names=tuple(all_names),
            out_names=tuple(out_names),
            lowering_input_output_aliases=(),
            sim_require_finite=True,
            sim_require_nnan=True,
            nc=nc,
        )
        return tuple(outs)

    devices = jax.devices()[:N_CORES]
    mesh = Mesh(np.asarray(devices), ("core",))
    spec = PartitionSpec("core")
    sharding = NamedSharding(mesh, spec)
    fn = jax.jit(
        shard_map(_body, mesh=mesh, in_specs=(spec,) * (n_params + n_outs),
                  out_specs=(spec,) * n_outs, check_rep=False),
        keep_unused=True)

    zeros_dev = [jax.device_put(
        np.zeros((N_CORES * z.shape[0], *z.shape[1:]), z.dtype), sharding)
        for z in zero_outs]

    meta = dict(in_names=in_names, out_names=out_names, out_avals=out_avals,
                sharding=sharding, zeros_dev=zeros_dev, jax=jax)
    _CACHE["fn"] = fn
    _CACHE["meta"] = meta
    return fn, meta


def kernel(x, w_qkv, w_proj, sink_logit):
    x = np.asarray(x, dtype=np.float32)
    w_qkv = np.asarray(w_qkv, dtype=np.float32)
    w_proj = np.asarray(w_proj, dtype=np.float32)
    sink_logit = np.asarray(sink_logit, dtype=np.float32)

    fn, meta = _get_runner()
    jax = meta["jax"]

    in_maps = [_make_core_inputs(x, w_qkv, w_proj, sink_logit, core)
               for core in range(N_CORES)]
    concat_in = [
        jax.device_put(
            np.concatenate([in_maps[c][nm] for c in range(N_CORES)], axis=0),
            meta["sharding"])
        for nm in meta["in_names"]]

    out_arrs = fn(*concat_in, *meta["zeros_dev"])
    jax.block_until_ready(out_arrs)

    i_out = meta["out_names"].index("out")
    per_core = np.asarray(out_arrs[i_out]).reshape(N_CORES, T, C)

    out = np.zeros((B, T, C), np.float64)
    for core in range(N_CORES):
        out[core // 4] += per_core[core].astype(np.float64)
    return out.astype(np.float32)
